# revision 32
# baseline (speedup 1.0000x reference)
"""Causal self-attention (RoPE + qk-RMS-norm) Trainium2 kernel.

Sharding: 8 cores = 2 batches x 4 head-groups (tensor-parallel over heads,
data-parallel over batch). Each core computes its head-group's attention and
a row-parallel partial of the output projection; the host sums the 4
per-group partials per batch (the all-reduce of row-parallel sharding).

Per-core layout: Q.T/K.T computed directly in [d, t] (no transposes),
V in [t, d]. Transposed flash attention: S.T = K @ Q.T so P.T feeds the
PV matmul directly; softmax has no max-subtraction (RMS-normed scores are
bounded by sqrt(D)); column sums via ones-matmul; 1/sum deferred to Y.T.
Tokens are processed in two causal passes (halves of T) to fit SBUF.

Schedule (v2): per head h the PE runs QKproj(h) -> attention(h-1) ->
rope/norm(h), so the rope-product chain (PSUM evict -> DVE/gpsimd products)
of head h hides under attention(h-1) instead of stalling the PE. V
projections are emitted as PE filler right where DMA/chain latency would
otherwise bubble (pass starts). The q-norm row is produced by a PE
transpose + SBUF->SBUF DMA gather (8 packets) instead of a DRAM bounce
(1024 4B packets), and both softmax-normalize broadcasts use gpsimd
partition_broadcast instead of PE ones-matmuls.
"""

import functools

import numpy as np

B, T, C, H, D = 2, 2048, 1280, 10, 128
EPS = 1e-5
NHL = 3  # head slots per core (padded)
N_CORES = 8
NHALF = 2  # causal passes over T
# per-batch head groups (4th group padded with zero heads)
GROUPS = [[0, 1, 2], [3, 4, 5], [6, 7, 8], [9]]


def _emit(nc, tile, mybir, T, C, D, NHL, eps):
    F32 = mybir.dt.float32
    BF16 = mybir.dt.bfloat16
    I32 = mybir.dt.int32
    ActF = mybir.ActivationFunctionType
    Alu = mybir.AluOpType
    CCH = C // 128  # contraction chunks
    TBN = T // 128  # 128-token blocks
    T2 = T // NHALF  # tokens per pass
    TB2 = T2 // 128
    Q42 = T2 // 512  # q supertiles per pass
    HD = NHL * D
    couts = []
    off = 0
    while off < C:
        w = min(512, C - off)
        couts.append((off, w))
        off += w

    xt = nc.dram_tensor("xt", [C, T], BF16, kind="ExternalInput")
    wqt = nc.dram_tensor("wqt", [C, HD], BF16, kind="ExternalInput")
    wkt = nc.dram_tensor("wkt", [C, HD], BF16, kind="ExternalInput")
    wvt = nc.dram_tensor("wvt", [C, HD], BF16, kind="ExternalInput")
    wpt = nc.dram_tensor("wpt", [HD, C], BF16, kind="ExternalInput")
    cs = nc.dram_tensor("cs", [D, T], BF16, kind="ExternalInput")
    sc = nc.dram_tensor("sc", [D, T], BF16, kind="ExternalInput")
    # host-precomputed constants: [tri01 | ma | mb | ident]
    kconsts = nc.dram_tensor("kconsts", [128, 512], BF16, kind="ExternalInput")
    out = nc.dram_tensor("out", [T, C], BF16, kind="ExternalOutput")

    from contextlib import ExitStack

    with ExitStack() as ctx:
        ctx.enter_context(nc.allow_low_precision(reason="bf16 operands"))
        tc = ctx.enter_context(tile.TileContext(nc))
        pool = lambda n, b, **kw: ctx.enter_context(tc.tile_pool(name=n, bufs=b, **kw))
        per = pool("persist", 1)
        wvp = pool("wv", 1)
        wqkp = pool("wqk", 1)
        wptp = pool("wpt", 1)
        xtp = pool("xt", 2)
        qtp = pool("qt", 2)
        qsp = pool("qs", 1)
        ytp = pool("yt", 2)
        tmp = pool("tmp", 2)
        sqp = pool("sqp", 1)
        ptp = pool("ptp", 3)
        rows = pool("rows", 2)
        oev = pool("oev", 2)
        bqp = pool("bqp", 2)
        psmm = pool("psmm", 2, space="PSUM")
        psacc = pool("psacc", 2, space="PSUM")
        psrow = pool("psrow", 2, space="PSUM")

        # ---- constants ----
        ones_f = per.tile([128, 128], F32, tag="onf")
        nc.vector.memset(ones_f[:], 1.0)
        ones_col = per.tile([128, 1], BF16, tag="onc")
        nc.scalar.copy(ones_col[:], ones_f[:, 0:1])
        # constants tile: [tri01 | ma | mb | ident] DMA'd from the host
        kcon = per.tile([128, 512], BF16, tag="kcon")
        nc.sync.dma_start(kcon[:], kconsts[:, :])
        tri01 = kcon[:, 0:128]
        ma = kcon[:, 128:256]
        mb = kcon[:, 256:384]
        ident = kcon[:, 384:512]

        # PE warm-up: dummy accumulating matmuls during the initial DMA ramp
        warm = nc.dram_tensor("warm", [1, 512], F32, kind="ExternalOutput")
        wrhs = per.tile([128, 512], BF16, tag="wrhs")
        nc.vector.memset(wrhs[:], 1.0)
        wps = psrow.tile([1, 512], F32, tag="row", name="warmps")
        NWARM = 24
        for i in range(NWARM):
            nc.tensor.matmul(
                wps[:], ones_col[:], wrhs[:], start=(i == 0), stop=(i == NWARM - 1)
            )
        wsb = rows.tile([1, 512], F32, tag="rw", name="warmsb")
        nc.vector.tensor_copy(wsb[:], wps[:])
        nc.sync.dma_start(warm[:], wsb[:])

        wv = []
        wqr = []
        wkr = []
        # V for all heads/all tokens: [tk-part, tb, h, d]
        v_t = per.tile([128, TBN, NHL, D], BF16, tag="v")
        # K.T per head, all tokens
        ktr = [per.tile([128, T], BF16, tag=f"ktr{h}", name=f"ktr{h}")
               for h in range(NHL)]
        rk_cols = [per.tile([128, TBN], F32, tag=f"rkc{h}", name=f"rkc{h}")
                   for h in range(NHL)]

        # output-projection weights (resident); loaded late (first needed at
        # the pass-0 output projection ~150us in) so the transfers don't
        # compete with the startup x/wv/wq/wk loads for HBM bandwidth
        wp = {}

        def load_wp():
            # on gpsimd (sync is reserved for the latency-critical rwr DMAs)
            for hh in range(NHL):
                for ci, (co, cw) in enumerate(couts):
                    t = wptp.tile([128, cw], BF16, tag=f"wp{hh}_{ci}")
                    nc.gpsimd.dma_start(
                        t[:], wpt[hh * 128 : (hh + 1) * 128, co : co + cw]
                    )
                    wp[(hh, ci)] = t

        def emit_qapply(qtn, rwr):
            """Deferred q-norm: broadcast the rsqrt row over partitions via
            gpsimd partition_broadcast, then scale qtn in place (all-bf16
            DVE muls run in 2x mode)."""
            bqt = bqp.tile([128, T2], BF16, tag="bqt")
            nc.gpsimd.partition_broadcast(bqt[:], rwr[:])
            for q4 in range(Q42):
                lsl = slice(q4 * 512, (q4 + 1) * 512)
                nc.vector.tensor_mul(qtn[:, lsl], qtn[:, lsl], bqt[:, lsl])

        def emit_attention(hf, h, qtn, ytn, mid_cb=None):
            """Attention for head h over this pass's q supertiles.
            kb-outer (K/V stationary reuse); st/exp run one kb ahead of
            PV/colsum so the in-order PE queue never waits on ACT.
            Normalize (1/colsum) is emitted inline per q4 as soon as its
            last PV lands -- pure DVE/gpsimd work, frees the PSUM
            accumulator immediately."""
            gq4s = [hf * Q42 + q4 for q4 in range(Q42)]
            csrs = []
            yts = [psacc.tile([128, 512], F32, tag="acc", name=f"yt{q4}")
                   for q4 in range(Q42)]

            def normalize_q4(q4):
                lsl = slice(q4 * 512, (q4 + 1) * 512)
                # reciprocal on the [1,512] row, then gpsimd broadcast
                rr = rows.tile([1, 512], F32, tag="rr", name="rr", bufs=2)
                nc.vector.reciprocal_approx_fast(rr[:], csrs[q4][:])
                rr8 = rows.tile([1, 512], BF16, tag="rr8", name="rr8", bufs=2)
                nc.vector.tensor_copy(rr8[:], rr[:])
                bcb = bqp.tile([128, 512], BF16, tag="bcb", bufs=2)
                nc.gpsimd.partition_broadcast(bcb[:], rr8[:])
                nc.vector.tensor_mul(ytn[:, h, lsl], yts[q4][:], bcb[:])
            # P column-sum accumulator (DVE bf16: 2x mode, light queue load)
            pacc = sqp.tile([128, Q42 * 512], BF16, tag="pacc", bufs=2)
            kbmax = 4 * (gq4s[-1] + 1)
            LA = 2  # st/exp run this many kb steps ahead of PV
            pts = {}  # kb -> pair pt tile awaiting PV
            for kb in range(kbmax + LA):
                if kb < kbmax:
                    active = [q4 for q4 in range(Q42) if kb <= 4 * gq4s[q4] + 3]
                    q0 = active[0]
                    j0 = kb - 4 * gq4s[q0]
                    st = psmm.tile([128, Q42 * 512], F32, tag="mm", name="st")
                    for q4 in active:
                        # strictly-above-diagonal q columns of the first
                        # active q4 are never needed: narrow the S matmul
                        a0 = q4 * 512 + (j0 * 128 if q4 == q0 and j0 > 0 else 0)
                        nc.tensor.matmul(
                            st[:, a0 : (q4 + 1) * 512],
                            ktr[h][:, kb * 128 : (kb + 1) * 128],
                            qtn[:, a0 : (q4 + 1) * 512],
                            start=True, stop=True,
                        )
                    pt = ptp.tile([128, Q42 * 512], BF16, tag="pt")
                    # one exp over the contiguous valid span of all active q4s
                    lo = q0 * 512 + (j0 * 128 if j0 > 0 else 0)
                    hi = (active[-1] + 1) * 512
                    nc.scalar.activation(
                        pt[:, lo:hi], st[:, lo:hi], ActF.Exp,
                        scale=rk_cols[h][:, kb : kb + 1],
                    )
                    if 0 <= j0 <= 3:
                        dg = slice(q0 * 512 + j0 * 128, q0 * 512 + (j0 + 1) * 128)
                        nc.vector.tensor_mul(pt[:, dg], pt[:, dg], tri01)
                    for q4 in active:
                        a0 = q4 * 512 + (j0 * 128 if q4 == q0 and j0 > 0 else 0)
                        lsl = slice(a0, (q4 + 1) * 512)
                        if kb == 0:
                            nc.vector.tensor_copy(pacc[:, lsl], pt[:, lsl])
                        else:
                            nc.vector.tensor_add(
                                pacc[:, lsl], pacc[:, lsl], pt[:, lsl]
                            )
                        # per-q4 column sum as soon as its pacc is final, so
                        # the pass-end chain isn't serialized behind the
                        # whole DVE queue
                        if kb == 4 * gq4s[q4] + 3:
                            csum = psrow.tile([1, 512], F32, tag="row",
                                              name=f"cs{q4}")
                            nc.tensor.matmul(
                                csum[:], ones_col[:],
                                pacc[:, q4 * 512 : (q4 + 1) * 512],
                                start=True, stop=True,
                            )
                            csr = rows.tile([1, 512], F32, tag="csr",
                                            name="csr", bufs=4)
                            nc.vector.tensor_copy(csr[:], csum[:])
                            csrs.append(csr)
                    pts[kb] = pt
                if kb == 4 and mid_cb is not None:
                    # deferred PE work (the next head's rsqrt transpose)
                    # slotted a few kb into this attention, when its DVE
                    # input has certainly landed
                    mid_cb()
                    mid_cb = None
                if kb >= LA:
                    pkb = kb - LA
                    pt = pts.pop(pkb)
                    for q4 in range(Q42):
                        gq4 = gq4s[q4]
                        last_kb = 4 * gq4 + 3
                        if pkb > last_kb:
                            continue
                        jp = pkb - 4 * gq4
                        w0 = jp * 128 if jp > 0 else 0
                        nc.tensor.matmul(
                            yts[q4][:, w0:],
                            v_t[:, pkb, h, :],
                            pt[:, q4 * 512 + w0 : (q4 + 1) * 512],
                            start=(pkb == 0), stop=(pkb == last_kb),
                        )
                        if pkb == last_kb:
                            normalize_q4(q4)

        pending = None  # deferred attention emitter for the previous head
        pending_qapply = None  # deferred q-norm apply for the previous head
        pending_oproj = None  # deferred output projection for the prev pass
        pending_transpose = None  # deferred rsqrt-row transpose

        def load_xc(hf_):
            # pass-1 prefetch: one full-width descriptor per chunk on
            # gpsimd (no urgency; fewer descriptors keeps gpsimd free for
            # the partition_broadcasts)
            toff_ = hf_ * T2
            xcl = []
            for c in range(CCH):
                t = xtp.tile([128, T2], BF16, tag=f"x{c}", name=f"x{c}")
                nc.gpsimd.dma_start(
                    t[:], xt[c * 128 : (c + 1) * 128, toff_ : toff_ + T2]
                )
                xcl.append(t)
            return xcl

        xc_next = None
        for hf in range(NHALF):
            toff = hf * T2
            # ---- per-pass cos/sin (stacked) ----
            cs_t = qtp.tile([D, T2], BF16, tag="cs", bufs=1)
            sc_t = qtp.tile([D, T2], BF16, tag="sc", bufs=1)
            nc.sync.dma_start(cs_t[:], cs[:, toff : toff + T2])
            nc.sync.dma_start(sc_t[:], sc[:, toff : toff + T2])
            # ---- x.T chunks: loaded here for pass 0, prefetched mid-pass-0
            # (double-buffered) for pass 1 ----
            if xc_next is not None:
                xc = xc_next
                xc_next = None
            else:
                # pass-0 startup, in consumption-priority order: x first
                # halves, wv, x second halves, wq, wk -- alternating the two
                # DMA-capable compute engines throughout.  wq after x second
                # halves: V-proj (first half) + warmup cover the ramp, and
                # QK-proj h0 only starts once both x halves are resident.
                xc = []
                for c in range(CCH):
                    t = xtp.tile([128, T2], BF16, tag=f"x{c}", name=f"x{c}")
                    xc.append(t)

                def xc_issue(half):
                    tsl = slice(half * (T2 // 2), (half + 1) * (T2 // 2))
                    for c in range(CCH):
                        (nc.gpsimd if c % 2 == 0 else nc.scalar).dma_start(
                            xc[c][:, tsl],
                            xt[c * 128 : (c + 1) * 128,
                               toff + half * (T2 // 2) :
                               toff + (half + 1) * (T2 // 2)],
                        )

                # wv interleaved with x half-0 (both needed by the first
                # V-proj blocks), then wq, wk, x half-1: QK-proj h0 emits
                # its q4-0 slices first, which need only x half-0
                tsl0 = slice(0, T2 // 2)
                for c in range(CCH):
                    nc.gpsimd.dma_start(
                        xc[c][:, tsl0],
                        xt[c * 128 : (c + 1) * 128, toff : toff + T2 // 2],
                    )
                    t = wvp.tile([128, HD], BF16, tag=f"wv{c}")
                    nc.scalar.dma_start(t[:], wvt[c * 128 : (c + 1) * 128, :])
                    wv.append(t)
                for c in range(CCH):
                    tq = wqkp.tile([128, HD], BF16, tag=f"wq{c}")
                    (nc.gpsimd if c % 2 == 0 else nc.scalar).dma_start(
                        tq[:], wqt[c * 128 : (c + 1) * 128, :]
                    )
                    wqr.append(tq)
                for c in range(CCH):
                    tk = wqkp.tile([128, HD], BF16, tag=f"wk{c}")
                    (nc.gpsimd if c % 2 == 0 else nc.scalar).dma_start(
                        tk[:], wkt[c * 128 : (c + 1) * 128, :]
                    )
                    wkr.append(tk)
                xc_issue(1)
                # dummy broadcast: loads the gpsimd custom-ISA microcode
                # library (~7us) during the DMA ramp, after the startup DMA
                # issues so descriptors aren't delayed behind it
                prime = bqp.tile([128, 512], BF16, tag="bcb")
                nc.gpsimd.partition_broadcast(prime[:], wrhs[0:1, :])

            def vproj_tbs(tb_lo, tb_hi, hf=hf, xc=xc):
                for tb in range(tb_lo, tb_hi):
                    gtb = hf * TB2 + tb
                    vp = psmm.tile([128, HD], F32, tag="mm", name="vp")
                    for c in range(CCH):
                        nc.tensor.matmul(
                            vp[:],
                            xc[c][:, tb * 128 : (tb + 1) * 128],
                            wv[c][:],
                            start=(c == 0), stop=(c == CCH - 1),
                        )
                    nc.vector.tensor_copy(v_t[:, gtb, :, :], vp[:])

            if hf == 0:
                # first half now; second half after QKproj(h0) as PE filler
                # while the h0 rope-product chain completes
                vproj_tbs(0, TB2 // 2)
                # second warm-up batch: soaks the remaining DMA ramp (wq
                # arrives after x half-0 + wv) and keeps the PE p-state high
                wps2 = psrow.tile([1, 512], F32, tag="row", name="warmps2")
                for i in range(12):
                    nc.tensor.matmul(
                        wps2[:], ones_col[:], wrhs[:], start=(i == 0),
                        stop=(i == 11),
                    )
                wsb2 = rows.tile([1, 512], F32, tag="rw", name="warmsb2")
                nc.vector.tensor_copy(wsb2[:], wps2[:])
                nc.sync.dma_start(warm[:], wsb2[:])

            # Y.T for this pass (all heads)
            ytn = ytp.tile([128, NHL, T2], BF16, tag="ytn")

            for h in range(NHL):
                # ---- Q/K projections into PSUM, evicted early to SBUF ----
                hds = slice(h * D, (h + 1) * D)
                qsb = {}
                qpst = {}
                for isq in range(2):
                    qpst[isq] = psmm.tile(
                        [128, Q42 * 512], F32, tag="mm", name="qps"
                    )
                # q4-0 slices of q then k first (they need only x half-0 at
                # startup), then the q4-1 slices; evict each slice as soon
                # as its chain completes
                for q4 in range(Q42):
                    if q4 == 1 and pending_transpose is not None:
                        # previous head's rsqrt transpose: mid-QKproj the
                        # PE reaches it ~4us in, when rqc has long landed,
                        # and the rwr->broadcast chain still finishes well
                        # before that head's attention needs it
                        pending_transpose()
                        pending_transpose = None
                    for isq, wt in enumerate((wqr, wkr)):
                        qps = qpst[isq]
                        for c in range(CCH):
                            nc.tensor.matmul(
                                qps[:, q4 * 512 : (q4 + 1) * 512],
                                wt[c][:, hds],
                                xc[c][:, q4 * 512 : (q4 + 1) * 512],
                                start=(c == 0), stop=(c == CCH - 1),
                            )
                        sb = qsp.tile([128, 512], BF16, tag=f"qs{isq}{q4}")
                        # q evictions on DVE (feed the critical rope->norm
                        # chain), k evictions on scalar to split queue load
                        if isq == 0:
                            nc.vector.tensor_copy(
                                sb[:], qps[:, q4 * 512 : (q4 + 1) * 512]
                            )
                        else:
                            nc.scalar.copy(
                                sb[:], qps[:, q4 * 512 : (q4 + 1) * 512]
                            )
                        qsb[(isq, q4)] = sb

                # PE filler between QKproj(h) and the attention/rope below:
                # V-proj second half (pass 0 also rides out the DMA ramp;
                # pass 1's first half was emitted at the pass-0 flush)
                if h == 0:
                    vproj_tbs(TB2 // 2, TB2)

                qtn = qtp.tile([128, T2], BF16, tag="qtn")

                # deferred q-norm apply for the previous head (its rsqrt row
                # landed during this head's QK projection), then its
                # attention; the rope-product chain for THIS head below
                # overlaps that attention on the PE
                if pending_qapply is not None:
                    pending_qapply()
                    pending_qapply = None

                # ---- rope products, phase A (emitted before the attention
                # so they sit early in the DVE/gpsimd queues) ----
                tprod = {}
                for isq in range(2):
                    for q4 in range(Q42):
                        # all products on DVE: gpsimd runs only DMA issues +
                        # partition_broadcast, so its custom-ISA microcode
                        # library is loaded once and never swapped (a swap
                        # costs ~6-7us of gpsimd downtime)
                        eng = nc.vector
                        qp = qsb[(isq, q4)]
                        lsl4 = slice(q4 * 512, (q4 + 1) * 512)
                        t1 = tmp.tile([128, 512], BF16, tag=f"t1{isq}{q4}")
                        t2 = tmp.tile([128, 512], BF16, tag=f"t2{isq}{q4}")
                        eng.tensor_mul(t1[:], qp[:], cs_t[:, lsl4])
                        eng.tensor_mul(t2[:], qp[:], sc_t[:, lsl4])
                        tprod[(isq, q4)] = (t1, t2)

                # bulk DMA-issue bursts go on gpsimd AFTER the qapply
                # broadcast is queued, and split across head sections so
                # no single burst delays the next head's bqt broadcast
                if hf == 0 and h == 0:
                    load_wp()
                if hf == 0 and h == 1:
                    xc_next = load_xc(hf + 1)

                # the pass-0 output projection runs here in pass-1 h0's
                # attention slot: its PE work covers the h0 rope-product
                # chain, and DVE saw h0's products queued first
                if pending_oproj is not None:
                    pending_oproj()
                    pending_oproj = None

                if pending is not None and h < NHL - 1:
                    pending()
                    pending = None

                # ---- rope + norm ----
                # Sum-of-squares lands as per-128-block COLUMNS (tiny N=1
                # matmuls), rsqrt is a quake-style bit-trick + 2 Newton steps
                # on DVE -- no scalar Sqrt/Ln, so the Exp act table is never
                # swapped out.
                nrm = psrow.tile([128, 2 * Q42 * 4], F32, tag="row", name="nrm")
                rope_io = [(qtn, 0), (ktr[h], toff)]
                for isq, (dst, doff) in enumerate(rope_io):
                    # phase B: rope matmuls + evictions (q side on DVE: its
                    # consumer S runs behind a DVE sem anyway and the ACT
                    # queue is exp-heavy in pass 1)
                    for q4 in range(Q42):
                        dsl = slice(doff + q4 * 512, doff + (q4 + 1) * 512)
                        t1, t2 = tprod[(isq, q4)]
                        rp = psmm.tile([128, 512], F32, tag="mm", name="rp")
                        nc.tensor.matmul(rp[:], ma, t1[:], start=True, stop=False)
                        nc.tensor.matmul(rp[:], mb, t2[:], start=False, stop=True)
                        if isq == 0:
                            nc.vector.tensor_copy(dst[:, dsl], rp[:])
                        else:
                            nc.scalar.copy(dst[:, dsl], rp[:])
                    # phase C: squares (on ACT, right behind the rp evicts
                    # in its queue -- keeps DVE free for the rsqrt chain)
                    # + per-block column reduces
                    for q4 in range(Q42):
                        dsl = slice(doff + q4 * 512, doff + (q4 + 1) * 512)
                        sq = sqp.tile([128, 512], BF16, tag="sq")
                        nc.scalar.square(sq[:], dst[:, dsl])
                        for b in range(4):
                            co = isq * 8 + q4 * 4 + b
                            nc.tensor.matmul(
                                nrm[:, co : co + 1],
                                sq[:, b * 128 : (b + 1) * 128], ones_col[:],
                                start=True, stop=True,
                            )
                # rsqrt chain on [128, 16]: cols 0:8 = q (no eps; pad heads
                # get nonzero Wq host-side), cols 8:16 = k (ssk/D + eps)
                nsb = rows.tile([128, 16], F32, tag="nsb")
                nc.vector.tensor_copy(nsb[:, 0:8], nrm[:, 0:8])
                nc.vector.tensor_scalar(
                    nsb[:, 8:16], nrm[:, 8:16], 1.0 / D, float(eps),
                    op0=Alu.mult, op1=Alu.add,
                )
                ysb = rows.tile([128, 16], F32, tag="ysb")
                nsi = nsb[:].bitcast(I32)
                ysi = ysb[:].bitcast(I32)
                nc.vector.tensor_scalar(
                    ysi, nsi, 1, None, op0=Alu.logical_shift_right
                )
                nc.vector.tensor_scalar(
                    ysi, ysi, 0x5F3759DF, -1, op0=Alu.subtract, op1=Alu.mult
                )
                ntmp = rows.tile([128, 16], F32, tag="ntmp")
                for _ in range(2):
                    nc.vector.tensor_mul(ntmp[:], ysb[:], ysb[:])
                    nc.vector.tensor_mul(ntmp[:], ntmp[:], nsb[:])
                    nc.vector.tensor_scalar(
                        ntmp[:], ntmp[:], -0.5, 1.5, op0=Alu.mult, op1=Alu.add
                    )
                    nc.vector.tensor_mul(ysb[:], ysb[:], ntmp[:])
                # k: rsqrt columns drop straight into rk_cols (no transpose)
                nc.vector.tensor_copy(
                    rk_cols[h][:, hf * TB2 : (hf + 1) * TB2], ysb[:, 8:16]
                )
                rqc = rows.tile([128, 8], BF16, tag="rqc")
                nc.vector.tensor_copy(rqc[:], ysb[:, 0:8])

                # q: [128, 8] cols -> PE transpose -> [8, 128] -> local
                # SBUF->SBUF gather into a [1, T2] row (8 contiguous
                # packets).  Emission is deferred (pending_transpose) to a
                # PE-queue spot that is reached only after rqc has landed,
                # so the in-order PE never stalls waiting for the DVE chain.
                rwr_box = {}

                def emit_transpose(rqc=rqc, rwr_box=rwr_box):
                    rqt_ps = psrow.tile([8, 128], BF16, tag="row", name="rqt")
                    nc.tensor.matmul(
                        rqt_ps[:], rqc[:], ident, is_transpose=True,
                        start=True, stop=True,
                    )
                    rqt = rows.tile([8, 128], BF16, tag="rqts", name="rqts")
                    nc.vector.tensor_copy(rqt[:], rqt_ps[:])
                    rwr = rows.tile([1, T2], BF16, tag="rwr", bufs=2)
                    nc.sync.dma_start(
                        rwr[0:1, :].rearrange("a (j p) -> a j p", p=128),
                        rqt[:],
                    )
                    rwr_box["rwr"] = rwr

                # last head: its consumer (the flush attention) comes right
                # after this section, so slot the transpose into the middle
                # of the previous head's attention instead
                if h == NHL - 1:
                    if pending is not None:
                        pending(mid_cb=emit_transpose)
                        pending = None
                    else:
                        emit_transpose()
                else:
                    pending_transpose = emit_transpose

                pending_qapply = (lambda qtn=qtn, box=rwr_box:
                                  emit_qapply(qtn, box["rwr"]))
                pending = (lambda hf=hf, h=h, qtn=qtn, ytn=ytn, mid_cb=None:
                           emit_attention(hf, h, qtn, ytn, mid_cb))

            # ---- pass flush: last head's attention; the output projection
            # is deferred into the next pass's h0 attention slot (its PE
            # work covers that head's rope chain, and the eviction CASTs
            # queue on DVE behind the products instead of ahead of them)
            if xc_next is not None:
                # pass-1 V-proj first half: PE filler covering the last
                # head's broadcast chain
                vproj_tbs(0, TB2 // 2, hf=hf + 1, xc=xc_next)
            if pending_qapply is not None:
                pending_qapply()
                pending_qapply = None
            if pending is not None:
                pending()
                pending = None

            def emit_oproj(ytn=ytn, toff=toff):
                for tb in range(TB2):
                    # fused eviction tile: one output DMA per token block
                    # (48 descriptors/pass -> 16, off the sync engine)
                    ot = oev.tile([128, C], BF16, tag="otb")
                    for ci, (co, cw) in enumerate(couts):
                        op = psacc.tile([128, cw], F32, tag="acc", name="op")
                        for hh in range(NHL):
                            nc.tensor.matmul(
                                op[:],
                                ytn[:, hh, tb * 128 : (tb + 1) * 128],
                                wp[(hh, ci)][:],
                                start=(hh == 0), stop=(hh == NHL - 1),
                            )
                        if (tb * len(couts) + ci) % 2 == 0:
                            nc.vector.tensor_copy(ot[:, co : co + cw], op[:])
                        else:
                            nc.scalar.copy(ot[:, co : co + cw], op[:])
                    nc.gpsimd.dma_start(
                        out[toff + tb * 128 : toff + (tb + 1) * 128, :], ot[:]
                    )

            pending_oproj = emit_oproj
        if pending_oproj is not None:
            pending_oproj()
    return nc


@functools.lru_cache(maxsize=4)
def _build(T_=T, C_=C, D_=D, NHL_=NHL, eps=EPS):
    import concourse.bacc as bacc
    import concourse.tile as tile
    from concourse import mybir

    nc = bacc.Bacc("TRN2", target_bir_lowering=False)
    _emit(nc, tile, mybir, T_, C_, D_, NHL_, eps)
    nc.compile()
    return nc


def _shard(x, cos, sin, Wq, Wk, Wv, Wproj):
    """Build the 8 per-core input maps."""
    import ml_dtypes

    BF = ml_dtypes.bfloat16
    HD = NHL * D
    cosT = np.ascontiguousarray(cos[0, 0].T.astype(np.float32))  # [64, T]
    sinT = np.ascontiguousarray(sin[0, 0].T.astype(np.float32))
    cs = np.concatenate([cosT, sinT], axis=0).astype(BF)  # [128, T]
    sc = np.concatenate([sinT, cosT], axis=0).astype(BF)

    # [tri01 | ma | mb | ident] constants (see _emit)
    kc = np.zeros((128, 512), np.float32)
    kc[:, 0:128] = np.triu(np.ones((128, 128), np.float32))
    for j in range(64):
        kc[j, 128 + j] = 1.0
        kc[64 + j, 128 + j] = 1.0
        kc[j, 256 + 64 + j] = -1.0
        kc[64 + j, 256 + 64 + j] = 1.0
    kc[:, 384:512] = np.eye(128, dtype=np.float32)
    kc = kc.astype(BF)

    def head_rows(W, heads, pad=0.0):
        rows = np.full((HD, C), pad, np.float32)
        for i, h in enumerate(heads):
            rows[i * D : (i + 1) * D] = W[h * D : (h + 1) * D]
        return rows

    in_maps = []
    for b in range(B):
        xtb = np.ascontiguousarray(x[b].T).astype(BF)  # [C, T]
        for heads in GROUPS:
            wq = np.ascontiguousarray(head_rows(Wq, heads, pad=0.01).T).astype(BF)
            wk = np.ascontiguousarray(head_rows(Wk, heads).T).astype(BF)
            wv = np.ascontiguousarray(head_rows(Wv, heads).T).astype(BF)
            # Wproj columns for these heads, transposed: [HD, C]
            wp = np.zeros((HD, C), np.float32)
            for i, h in enumerate(heads):
                wp[i * D : (i + 1) * D] = Wproj[:, h * D : (h + 1) * D].T
            in_maps.append(
                {"xt": xtb, "wqt": wq, "wkt": wk, "wvt": wv,
                 "wpt": wp.astype(BF), "cs": cs, "sc": sc, "kconsts": kc}
            )
    return in_maps


def _gather(results):
    y = np.zeros((B, T, C), np.float32)
    for b in range(B):
        for g in range(len(GROUPS)):
            y[b] += results[b * len(GROUPS) + g]["out"].astype(np.float32)
    return y


def _run(in_maps, trace=False):
    from concourse.bass_utils import run_bass_kernel_spmd

    nc = _build()
    return run_bass_kernel_spmd(
        nc, in_maps, core_ids=list(range(N_CORES)), trace=trace
    )


def kernel(x, cos, sin, Wq, Wk, Wv, Wproj):
    ins = _shard(
        np.asarray(x), np.asarray(cos), np.asarray(sin),
        np.asarray(Wq), np.asarray(Wk), np.asarray(Wv), np.asarray(Wproj),
    )
    res = _run(ins, trace=False)
    return _gather(res.results)


def run_traced(x, cos, sin, Wq, Wk, Wv, Wproj):
    ins = _shard(
        np.asarray(x), np.asarray(cos), np.asarray(sin),
        np.asarray(Wq), np.asarray(Wk), np.asarray(Wv), np.asarray(Wproj),
    )
    res = _run(ins, trace=True)
    return _gather(res.results), res


# revision 33
# speedup vs baseline: 1.0061x; 1.0061x over previous
"""Causal self-attention (RoPE + qk-RMS-norm) Trainium2 kernel.

Sharding: 8 cores = 2 batches x 4 head-groups (tensor-parallel over heads,
data-parallel over batch). Each core computes its head-group's attention and
a row-parallel partial of the output projection; the host sums the 4
per-group partials per batch (the all-reduce of row-parallel sharding).

Per-core layout: Q.T/K.T computed directly in [d, t] (no transposes),
V in [t, d]. Transposed flash attention: S.T = K @ Q.T so P.T feeds the
PV matmul directly; softmax has no max-subtraction (RMS-normed scores are
bounded by sqrt(D)); column sums via ones-matmul; 1/sum deferred to Y.T.
Tokens are processed in two causal passes (halves of T) to fit SBUF.

Schedule (v2): per head h the PE runs QKproj(h) -> attention(h-1) ->
rope/norm(h), so the rope-product chain (PSUM evict -> DVE/gpsimd products)
of head h hides under attention(h-1) instead of stalling the PE. V
projections are emitted as PE filler right where DMA/chain latency would
otherwise bubble (pass starts). The q-norm row is produced by a PE
transpose + SBUF->SBUF DMA gather (8 packets) instead of a DRAM bounce
(1024 4B packets), and both softmax-normalize broadcasts use gpsimd
partition_broadcast instead of PE ones-matmuls.
"""

import functools

import numpy as np

B, T, C, H, D = 2, 2048, 1280, 10, 128
EPS = 1e-5
NHL = 3  # head slots per core (padded)
N_CORES = 8
NHALF = 2  # causal passes over T
# per-batch head groups (4th group padded with zero heads)
GROUPS = [[0, 1, 2], [3, 4, 5], [6, 7, 8], [9]]


def _emit(nc, tile, mybir, T, C, D, NHL, eps):
    F32 = mybir.dt.float32
    BF16 = mybir.dt.bfloat16
    I32 = mybir.dt.int32
    ActF = mybir.ActivationFunctionType
    Alu = mybir.AluOpType
    CCH = C // 128  # contraction chunks
    TBN = T // 128  # 128-token blocks
    T2 = T // NHALF  # tokens per pass
    TB2 = T2 // 128
    Q42 = T2 // 512  # q supertiles per pass
    HD = NHL * D
    couts = []
    off = 0
    while off < C:
        w = min(512, C - off)
        couts.append((off, w))
        off += w

    xt = nc.dram_tensor("xt", [C, T], BF16, kind="ExternalInput")
    wqt = nc.dram_tensor("wqt", [C, HD], BF16, kind="ExternalInput")
    wkt = nc.dram_tensor("wkt", [C, HD], BF16, kind="ExternalInput")
    wvt = nc.dram_tensor("wvt", [C, HD], BF16, kind="ExternalInput")
    wpt = nc.dram_tensor("wpt", [HD, C], BF16, kind="ExternalInput")
    cs = nc.dram_tensor("cs", [D, T], BF16, kind="ExternalInput")
    sc = nc.dram_tensor("sc", [D, T], BF16, kind="ExternalInput")
    # host-precomputed constants: [tri01 | ma | mb | ident]
    kconsts = nc.dram_tensor("kconsts", [128, 512], BF16, kind="ExternalInput")
    out = nc.dram_tensor("out", [T, C], BF16, kind="ExternalOutput")

    from contextlib import ExitStack

    with ExitStack() as ctx:
        ctx.enter_context(nc.allow_low_precision(reason="bf16 operands"))
        tc = ctx.enter_context(tile.TileContext(nc))
        pool = lambda n, b, **kw: ctx.enter_context(tc.tile_pool(name=n, bufs=b, **kw))
        per = pool("persist", 1)
        wvp = pool("wv", 1)
        wqkp = pool("wqk", 1)
        wptp = pool("wpt", 1)
        xtp = pool("xt", 2)
        qtp = pool("qt", 2)
        qsp = pool("qs", 1)
        ytp = pool("yt", 2)
        tmp = pool("tmp", 2)
        sqp = pool("sqp", 1)
        ptp = pool("ptp", 3)
        rows = pool("rows", 2)
        oev = pool("oev", 2)
        bqp = pool("bqp", 2)
        psmm = pool("psmm", 2, space="PSUM")
        psacc = pool("psacc", 2, space="PSUM")
        psrow = pool("psrow", 2, space="PSUM")

        # ---- constants ----
        ones_f = per.tile([128, 128], F32, tag="onf")
        nc.vector.memset(ones_f[:], 1.0)
        ones_col = per.tile([128, 1], BF16, tag="onc")
        nc.scalar.copy(ones_col[:], ones_f[:, 0:1])
        # constants tile: [tri01 | ma | mb | ident] DMA'd from the host
        kcon = per.tile([128, 512], BF16, tag="kcon")
        nc.sync.dma_start(kcon[:], kconsts[:, :])
        tri01 = kcon[:, 0:128]
        ma = kcon[:, 128:256]
        mb = kcon[:, 256:384]
        ident = kcon[:, 384:512]

        # PE warm-up: dummy accumulating matmuls during the initial DMA ramp
        warm = nc.dram_tensor("warm", [1, 512], F32, kind="ExternalOutput")
        wrhs = per.tile([128, 512], BF16, tag="wrhs")
        nc.vector.memset(wrhs[:], 1.0)
        wps = psrow.tile([1, 512], F32, tag="row", name="warmps")
        NWARM = 24
        for i in range(NWARM):
            nc.tensor.matmul(
                wps[:], ones_col[:], wrhs[:], start=(i == 0), stop=(i == NWARM - 1)
            )
        wsb = rows.tile([1, 512], F32, tag="rw", name="warmsb")
        nc.vector.tensor_copy(wsb[:], wps[:])
        nc.sync.dma_start(warm[:], wsb[:])

        wv = []
        wqr = []
        wkr = []
        # V for all heads/all tokens: [tk-part, tb, h, d]
        v_t = per.tile([128, TBN, NHL, D], BF16, tag="v")
        # K.T per head, all tokens
        ktr = [per.tile([128, T], BF16, tag=f"ktr{h}", name=f"ktr{h}")
               for h in range(NHL)]
        rk_cols = [per.tile([128, TBN], F32, tag=f"rkc{h}", name=f"rkc{h}")
                   for h in range(NHL)]

        # output-projection weights (resident); loaded late (first needed at
        # the pass-0 output projection ~150us in) so the transfers don't
        # compete with the startup x/wv/wq/wk loads for HBM bandwidth
        wp = {}

        def load_wp():
            # on gpsimd (sync is reserved for the latency-critical rwr DMAs)
            for hh in range(NHL):
                for ci, (co, cw) in enumerate(couts):
                    t = wptp.tile([128, cw], BF16, tag=f"wp{hh}_{ci}")
                    nc.gpsimd.dma_start(
                        t[:], wpt[hh * 128 : (hh + 1) * 128, co : co + cw]
                    )
                    wp[(hh, ci)] = t

        def emit_qapply(qtn, rwr):
            """Deferred q-norm: broadcast the rsqrt row over partitions via
            gpsimd partition_broadcast, then scale qtn in place (all-bf16
            DVE muls run in 2x mode)."""
            bqt = bqp.tile([128, T2], BF16, tag="bqt")
            nc.gpsimd.partition_broadcast(bqt[:], rwr[:])
            for q4 in range(Q42):
                lsl = slice(q4 * 512, (q4 + 1) * 512)
                nc.vector.tensor_mul(qtn[:, lsl], qtn[:, lsl], bqt[:, lsl])

        def emit_attention(hf, h, qtn, ytn, mid_cb=None):
            """Attention for head h over this pass's q supertiles.
            kb-outer (K/V stationary reuse); st/exp run one kb ahead of
            PV/colsum so the in-order PE queue never waits on ACT.
            Normalize (1/colsum) is emitted inline per q4 as soon as its
            last PV lands -- pure DVE/gpsimd work, frees the PSUM
            accumulator immediately."""
            gq4s = [hf * Q42 + q4 for q4 in range(Q42)]
            csrs = []
            yts = [psacc.tile([128, 512], F32, tag="acc", name=f"yt{q4}")
                   for q4 in range(Q42)]

            def normalize_q4(q4):
                lsl = slice(q4 * 512, (q4 + 1) * 512)
                # reciprocal on the [1,512] row, then gpsimd broadcast
                rr = rows.tile([1, 512], F32, tag="rr", name="rr", bufs=2)
                nc.vector.reciprocal_approx_fast(rr[:], csrs[q4][:])
                rr8 = rows.tile([1, 512], BF16, tag="rr8", name="rr8", bufs=2)
                nc.vector.tensor_copy(rr8[:], rr[:])
                bcb = bqp.tile([128, 512], BF16, tag="bcb", bufs=2)
                nc.gpsimd.partition_broadcast(bcb[:], rr8[:])
                nc.vector.tensor_mul(ytn[:, h, lsl], yts[q4][:], bcb[:])
            # P column-sum accumulator (DVE bf16: 2x mode, light queue load)
            pacc = sqp.tile([128, Q42 * 512], BF16, tag="pacc", bufs=2)
            kbmax = 4 * (gq4s[-1] + 1)
            LA = 2  # st/exp run this many kb steps ahead of PV
            pts = {}  # kb -> pair pt tile awaiting PV
            for kb in range(kbmax + LA):
                if kb < kbmax:
                    active = [q4 for q4 in range(Q42) if kb <= 4 * gq4s[q4] + 3]
                    q0 = active[0]
                    j0 = kb - 4 * gq4s[q0]
                    st = psmm.tile([128, Q42 * 512], F32, tag="mm", name="st")
                    for q4 in active:
                        # strictly-above-diagonal q columns of the first
                        # active q4 are never needed: narrow the S matmul
                        a0 = q4 * 512 + (j0 * 128 if q4 == q0 and j0 > 0 else 0)
                        nc.tensor.matmul(
                            st[:, a0 : (q4 + 1) * 512],
                            ktr[h][:, kb * 128 : (kb + 1) * 128],
                            qtn[:, a0 : (q4 + 1) * 512],
                            start=True, stop=True,
                        )
                    pt = ptp.tile([128, Q42 * 512], BF16, tag="pt")
                    # one exp over the contiguous valid span of all active q4s
                    lo = q0 * 512 + (j0 * 128 if j0 > 0 else 0)
                    hi = (active[-1] + 1) * 512
                    nc.scalar.activation(
                        pt[:, lo:hi], st[:, lo:hi], ActF.Exp,
                        scale=rk_cols[h][:, kb : kb + 1],
                    )
                    if 0 <= j0 <= 3:
                        dg = slice(q0 * 512 + j0 * 128, q0 * 512 + (j0 + 1) * 128)
                        nc.vector.tensor_mul(pt[:, dg], pt[:, dg], tri01)
                    for q4 in active:
                        a0 = q4 * 512 + (j0 * 128 if q4 == q0 and j0 > 0 else 0)
                        lsl = slice(a0, (q4 + 1) * 512)
                        if kb == 0:
                            nc.vector.tensor_copy(pacc[:, lsl], pt[:, lsl])
                        else:
                            nc.vector.tensor_add(
                                pacc[:, lsl], pacc[:, lsl], pt[:, lsl]
                            )
                        # per-q4 column sum as soon as its pacc is final, so
                        # the pass-end chain isn't serialized behind the
                        # whole DVE queue
                        if kb == 4 * gq4s[q4] + 3:
                            csum = psrow.tile([1, 512], F32, tag="row",
                                              name=f"cs{q4}")
                            nc.tensor.matmul(
                                csum[:], ones_col[:],
                                pacc[:, q4 * 512 : (q4 + 1) * 512],
                                start=True, stop=True,
                            )
                            csr = rows.tile([1, 512], F32, tag="csr",
                                            name="csr", bufs=4)
                            nc.vector.tensor_copy(csr[:], csum[:])
                            csrs.append(csr)
                    pts[kb] = pt
                if kb == 4 and mid_cb is not None:
                    # deferred PE work (the next head's rsqrt transpose)
                    # slotted a few kb into this attention, when its DVE
                    # input has certainly landed
                    mid_cb()
                    mid_cb = None
                if kb >= LA:
                    pkb = kb - LA
                    pt = pts.pop(pkb)
                    for q4 in range(Q42):
                        gq4 = gq4s[q4]
                        last_kb = 4 * gq4 + 3
                        if pkb > last_kb:
                            continue
                        jp = pkb - 4 * gq4
                        w0 = jp * 128 if jp > 0 else 0
                        nc.tensor.matmul(
                            yts[q4][:, w0:],
                            v_t[:, pkb, h, :],
                            pt[:, q4 * 512 + w0 : (q4 + 1) * 512],
                            start=(pkb == 0), stop=(pkb == last_kb),
                        )
                        if pkb == last_kb:
                            normalize_q4(q4)

        pending = None  # deferred attention emitter for the previous head
        pending_qapply = None  # deferred q-norm apply for the previous head
        pending_oproj = None  # deferred output projection for the prev pass
        pending_transpose = None  # deferred rsqrt-row transpose

        def load_xc(hf_):
            # pass-1 prefetch: one full-width descriptor per chunk on
            # gpsimd (no urgency; fewer descriptors keeps gpsimd free for
            # the partition_broadcasts)
            toff_ = hf_ * T2
            xcl = []
            for c in range(CCH):
                t = xtp.tile([128, T2], BF16, tag=f"x{c}", name=f"x{c}")
                nc.gpsimd.dma_start(
                    t[:], xt[c * 128 : (c + 1) * 128, toff_ : toff_ + T2]
                )
                xcl.append(t)
            return xcl

        xc_next = None
        for hf in range(NHALF):
            toff = hf * T2
            # ---- per-pass cos/sin (stacked) ----
            cs_t = qtp.tile([D, T2], BF16, tag="cs", bufs=1)
            sc_t = qtp.tile([D, T2], BF16, tag="sc", bufs=1)
            nc.sync.dma_start(cs_t[:], cs[:, toff : toff + T2])
            nc.sync.dma_start(sc_t[:], sc[:, toff : toff + T2])
            # ---- x.T chunks: loaded here for pass 0, prefetched mid-pass-0
            # (double-buffered) for pass 1 ----
            if xc_next is not None:
                xc = xc_next
                xc_next = None
            else:
                # pass-0 startup, in consumption-priority order: x first
                # halves, wv, x second halves, wq, wk -- alternating the two
                # DMA-capable compute engines throughout.  wq after x second
                # halves: V-proj (first half) + warmup cover the ramp, and
                # QK-proj h0 only starts once both x halves are resident.
                xc = []
                for c in range(CCH):
                    t = xtp.tile([128, T2], BF16, tag=f"x{c}", name=f"x{c}")
                    xc.append(t)

                def xc_issue(half):
                    tsl = slice(half * (T2 // 2), (half + 1) * (T2 // 2))
                    for c in range(CCH):
                        (nc.gpsimd if c % 2 == 0 else nc.scalar).dma_start(
                            xc[c][:, tsl],
                            xt[c * 128 : (c + 1) * 128,
                               toff + half * (T2 // 2) :
                               toff + (half + 1) * (T2 // 2)],
                        )

                # wv interleaved with x half-0 (both needed by the first
                # V-proj blocks), then wq, wk, x half-1: QK-proj h0 emits
                # its q4-0 slices first, which need only x half-0
                tsl0 = slice(0, T2 // 2)
                for c in range(CCH):
                    nc.gpsimd.dma_start(
                        xc[c][:, tsl0],
                        xt[c * 128 : (c + 1) * 128, toff : toff + T2 // 2],
                    )
                    t = wvp.tile([128, HD], BF16, tag=f"wv{c}")
                    nc.scalar.dma_start(t[:], wvt[c * 128 : (c + 1) * 128, :])
                    wv.append(t)
                for c in range(CCH):
                    tq = wqkp.tile([128, HD], BF16, tag=f"wq{c}")
                    (nc.gpsimd if c % 2 == 0 else nc.scalar).dma_start(
                        tq[:], wqt[c * 128 : (c + 1) * 128, :]
                    )
                    wqr.append(tq)
                for c in range(CCH):
                    tk = wqkp.tile([128, HD], BF16, tag=f"wk{c}")
                    (nc.gpsimd if c % 2 == 0 else nc.scalar).dma_start(
                        tk[:], wkt[c * 128 : (c + 1) * 128, :]
                    )
                    wkr.append(tk)
                xc_issue(1)
                # dummy broadcast: loads the gpsimd custom-ISA microcode
                # library (~7us) during the DMA ramp, after the startup DMA
                # issues so descriptors aren't delayed behind it
                prime = bqp.tile([128, 512], BF16, tag="bcb")
                nc.gpsimd.partition_broadcast(prime[:], wrhs[0:1, :])

            def vproj_tbs(tb_lo, tb_hi, hf=hf, xc=xc):
                for tb in range(tb_lo, tb_hi):
                    gtb = hf * TB2 + tb
                    vp = psmm.tile([128, HD], F32, tag="mm", name="vp")
                    for c in range(CCH):
                        nc.tensor.matmul(
                            vp[:],
                            xc[c][:, tb * 128 : (tb + 1) * 128],
                            wv[c][:],
                            start=(c == 0), stop=(c == CCH - 1),
                        )
                    nc.vector.tensor_copy(v_t[:, gtb, :, :], vp[:])

            if hf == 0:
                # first half now; second half after QKproj(h0) as PE filler
                # while the h0 rope-product chain completes
                vproj_tbs(0, TB2 // 2)
                # second warm-up batch: soaks the remaining DMA ramp (wq
                # arrives after x half-0 + wv) and keeps the PE p-state high
                wps2 = psrow.tile([1, 512], F32, tag="row", name="warmps2")
                for i in range(12):
                    nc.tensor.matmul(
                        wps2[:], ones_col[:], wrhs[:], start=(i == 0),
                        stop=(i == 11),
                    )
                wsb2 = rows.tile([1, 512], F32, tag="rw", name="warmsb2")
                nc.vector.tensor_copy(wsb2[:], wps2[:])
                nc.sync.dma_start(warm[:], wsb2[:])

            # Y.T for this pass (all heads)
            ytn = ytp.tile([128, NHL, T2], BF16, tag="ytn")

            for h in range(NHL):
                # ---- Q/K projections into PSUM, evicted early to SBUF ----
                hds = slice(h * D, (h + 1) * D)
                qsb = {}
                qpst = {}
                for isq in range(2):
                    qpst[isq] = psmm.tile(
                        [128, Q42 * 512], F32, tag="mm", name="qps"
                    )
                # q4-0 slices of q then k first (they need only x half-0 at
                # startup), then the q4-1 slices; evict each slice as soon
                # as its chain completes
                for q4 in range(Q42):
                    if q4 == 1 and pending_transpose is not None:
                        # previous head's rsqrt transpose: mid-QKproj the
                        # PE reaches it ~4us in, when rqc has long landed,
                        # and the rwr->broadcast chain still finishes well
                        # before that head's attention needs it
                        pending_transpose()
                        pending_transpose = None
                    for isq, wt in enumerate((wqr, wkr)):
                        qps = qpst[isq]
                        for c in range(CCH):
                            nc.tensor.matmul(
                                qps[:, q4 * 512 : (q4 + 1) * 512],
                                wt[c][:, hds],
                                xc[c][:, q4 * 512 : (q4 + 1) * 512],
                                start=(c == 0), stop=(c == CCH - 1),
                            )
                        sb = qsp.tile([128, 512], BF16, tag=f"qs{isq}{q4}")
                        # q evictions on DVE (feed the critical rope->norm
                        # chain), k evictions on scalar to split queue load
                        if isq == 0:
                            nc.vector.tensor_copy(
                                sb[:], qps[:, q4 * 512 : (q4 + 1) * 512]
                            )
                        else:
                            nc.scalar.copy(
                                sb[:], qps[:, q4 * 512 : (q4 + 1) * 512]
                            )
                        qsb[(isq, q4)] = sb

                # PE filler between QKproj(h) and the attention/rope below:
                # V-proj second half (pass 0 also rides out the DMA ramp;
                # pass 1's first half was emitted at the pass-0 flush)
                if h == 0:
                    vproj_tbs(TB2 // 2, TB2)

                qtn = qtp.tile([128, T2], BF16, tag="qtn")

                # deferred q-norm apply for the previous head (its rsqrt row
                # landed during this head's QK projection), then its
                # attention; the rope-product chain for THIS head below
                # overlaps that attention on the PE
                if pending_qapply is not None:
                    pending_qapply()
                    pending_qapply = None

                # ---- rope products, phase A (emitted before the attention
                # so they sit early in the DVE/gpsimd queues) ----
                tprod = {}
                for isq in range(2):
                    for q4 in range(Q42):
                        # all products on DVE: gpsimd runs only DMA issues +
                        # partition_broadcast, so its custom-ISA microcode
                        # library is loaded once and never swapped (a swap
                        # costs ~6-7us of gpsimd downtime)
                        eng = nc.vector
                        qp = qsb[(isq, q4)]
                        lsl4 = slice(q4 * 512, (q4 + 1) * 512)
                        t1 = tmp.tile([128, 512], BF16, tag=f"t1{isq}{q4}")
                        t2 = tmp.tile([128, 512], BF16, tag=f"t2{isq}{q4}")
                        eng.tensor_mul(t1[:], qp[:], cs_t[:, lsl4])
                        eng.tensor_mul(t2[:], qp[:], sc_t[:, lsl4])
                        tprod[(isq, q4)] = (t1, t2)

                # bulk DMA-issue bursts go on gpsimd AFTER the qapply
                # broadcast is queued, and split across head sections so
                # no single burst delays the next head's bqt broadcast
                if hf == 0 and h == 0:
                    load_wp()
                if hf == 0 and h == 1:
                    xc_next = load_xc(hf + 1)

                # the pass-0 output projection runs here in pass-1 h0's
                # attention slot: its PE work covers the h0 rope-product
                # chain, and DVE saw h0's products queued first
                if pending_oproj is not None:
                    pending_oproj()
                    pending_oproj = None

                if pending is not None and h < NHL - 1:
                    pending()
                    pending = None

                # ---- rope + norm ----
                # Sum-of-squares lands as per-128-block COLUMNS (tiny N=1
                # matmuls), rsqrt is a quake-style bit-trick + 2 Newton steps
                # on DVE -- no scalar Sqrt/Ln, so the Exp act table is never
                # swapped out.
                nrm = psrow.tile([128, 2 * Q42 * 4], F32, tag="row", name="nrm")
                rope_io = [(qtn, 0), (ktr[h], toff)]
                for isq, (dst, doff) in enumerate(rope_io):
                    # phase B: rope matmuls + evictions
                    for q4 in range(Q42):
                        dsl = slice(doff + q4 * 512, doff + (q4 + 1) * 512)
                        t1, t2 = tprod[(isq, q4)]
                        rp = psmm.tile([128, 512], F32, tag="mm", name="rp")
                        nc.tensor.matmul(rp[:], ma, t1[:], start=True, stop=False)
                        nc.tensor.matmul(rp[:], mb, t2[:], start=False, stop=True)
                        nc.scalar.copy(dst[:, dsl], rp[:])
                    # phase C: squares (on ACT, right behind the rp evicts
                    # in its queue -- keeps DVE free for the rsqrt chain)
                    # + per-block column reduces
                    for q4 in range(Q42):
                        dsl = slice(doff + q4 * 512, doff + (q4 + 1) * 512)
                        sq = sqp.tile([128, 512], BF16, tag="sq")
                        nc.scalar.square(sq[:], dst[:, dsl])
                        for b in range(4):
                            co = isq * 8 + q4 * 4 + b
                            nc.tensor.matmul(
                                nrm[:, co : co + 1],
                                sq[:, b * 128 : (b + 1) * 128], ones_col[:],
                                start=True, stop=True,
                            )
                # rsqrt chain on [128, 16]: cols 0:8 = q (no eps; pad heads
                # get nonzero Wq host-side), cols 8:16 = k (ssk/D + eps)
                nsb = rows.tile([128, 16], F32, tag="nsb")
                nc.vector.tensor_copy(nsb[:, 0:8], nrm[:, 0:8])
                nc.vector.tensor_scalar(
                    nsb[:, 8:16], nrm[:, 8:16], 1.0 / D, float(eps),
                    op0=Alu.mult, op1=Alu.add,
                )
                ysb = rows.tile([128, 16], F32, tag="ysb")
                nsi = nsb[:].bitcast(I32)
                ysi = ysb[:].bitcast(I32)
                nc.vector.tensor_scalar(
                    ysi, nsi, 1, None, op0=Alu.logical_shift_right
                )
                nc.vector.tensor_scalar(
                    ysi, ysi, 0x5F3759DF, -1, op0=Alu.subtract, op1=Alu.mult
                )
                ntmp = rows.tile([128, 16], F32, tag="ntmp")
                for _ in range(2):
                    nc.vector.tensor_mul(ntmp[:], ysb[:], ysb[:])
                    nc.vector.tensor_mul(ntmp[:], ntmp[:], nsb[:])
                    nc.vector.tensor_scalar(
                        ntmp[:], ntmp[:], -0.5, 1.5, op0=Alu.mult, op1=Alu.add
                    )
                    nc.vector.tensor_mul(ysb[:], ysb[:], ntmp[:])
                # k: rsqrt columns drop straight into rk_cols (no transpose)
                nc.vector.tensor_copy(
                    rk_cols[h][:, hf * TB2 : (hf + 1) * TB2], ysb[:, 8:16]
                )
                rqc = rows.tile([128, 8], BF16, tag="rqc")
                nc.vector.tensor_copy(rqc[:], ysb[:, 0:8])

                # q: [128, 8] cols -> PE transpose -> [8, 128] -> local
                # SBUF->SBUF gather into a [1, T2] row (8 contiguous
                # packets).  Emission is deferred (pending_transpose) to a
                # PE-queue spot that is reached only after rqc has landed,
                # so the in-order PE never stalls waiting for the DVE chain.
                rwr_box = {}

                def emit_transpose(rqc=rqc, rwr_box=rwr_box):
                    rqt_ps = psrow.tile([8, 128], BF16, tag="row", name="rqt")
                    nc.tensor.matmul(
                        rqt_ps[:], rqc[:], ident, is_transpose=True,
                        start=True, stop=True,
                    )
                    rqt = rows.tile([8, 128], BF16, tag="rqts", name="rqts")
                    nc.vector.tensor_copy(rqt[:], rqt_ps[:])
                    rwr = rows.tile([1, T2], BF16, tag="rwr", bufs=2)
                    nc.sync.dma_start(
                        rwr[0:1, :].rearrange("a (j p) -> a j p", p=128),
                        rqt[:],
                    )
                    rwr_box["rwr"] = rwr

                # last head: its consumer (the flush attention) comes right
                # after this section, so slot the transpose into the middle
                # of the previous head's attention instead
                if h == NHL - 1:
                    if pending is not None:
                        pending(mid_cb=emit_transpose)
                        pending = None
                    else:
                        emit_transpose()
                else:
                    pending_transpose = emit_transpose

                pending_qapply = (lambda qtn=qtn, box=rwr_box:
                                  emit_qapply(qtn, box["rwr"]))
                pending = (lambda hf=hf, h=h, qtn=qtn, ytn=ytn, mid_cb=None:
                           emit_attention(hf, h, qtn, ytn, mid_cb))

            # ---- pass flush: last head's attention; the output projection
            # is deferred into the next pass's h0 attention slot (its PE
            # work covers that head's rope chain, and the eviction CASTs
            # queue on DVE behind the products instead of ahead of them)
            if xc_next is not None:
                # pass-1 V-proj first half: PE filler covering the last
                # head's broadcast chain
                vproj_tbs(0, TB2 // 2, hf=hf + 1, xc=xc_next)
            if pending_qapply is not None:
                pending_qapply()
                pending_qapply = None
            if pending is not None:
                pending()
                pending = None

            def emit_oproj(ytn=ytn, toff=toff):
                for tb in range(TB2):
                    # fused eviction tile: one output DMA per token block
                    # (48 descriptors/pass -> 16, off the sync engine)
                    ot = oev.tile([128, C], BF16, tag="otb")
                    for ci, (co, cw) in enumerate(couts):
                        op = psacc.tile([128, cw], F32, tag="acc", name="op")
                        for hh in range(NHL):
                            nc.tensor.matmul(
                                op[:],
                                ytn[:, hh, tb * 128 : (tb + 1) * 128],
                                wp[(hh, ci)][:],
                                start=(hh == 0), stop=(hh == NHL - 1),
                            )
                        if (tb * len(couts) + ci) % 2 == 0:
                            nc.vector.tensor_copy(ot[:, co : co + cw], op[:])
                        else:
                            nc.scalar.copy(ot[:, co : co + cw], op[:])
                    nc.gpsimd.dma_start(
                        out[toff + tb * 128 : toff + (tb + 1) * 128, :], ot[:]
                    )

            pending_oproj = emit_oproj
        if pending_oproj is not None:
            pending_oproj()
    return nc


@functools.lru_cache(maxsize=4)
def _build(T_=T, C_=C, D_=D, NHL_=NHL, eps=EPS):
    import concourse.bacc as bacc
    import concourse.tile as tile
    from concourse import mybir

    nc = bacc.Bacc("TRN2", target_bir_lowering=False)
    _emit(nc, tile, mybir, T_, C_, D_, NHL_, eps)
    nc.compile()
    return nc


def _shard(x, cos, sin, Wq, Wk, Wv, Wproj):
    """Build the 8 per-core input maps."""
    import ml_dtypes

    BF = ml_dtypes.bfloat16
    HD = NHL * D
    cosT = np.ascontiguousarray(cos[0, 0].T.astype(np.float32))  # [64, T]
    sinT = np.ascontiguousarray(sin[0, 0].T.astype(np.float32))
    cs = np.concatenate([cosT, sinT], axis=0).astype(BF)  # [128, T]
    sc = np.concatenate([sinT, cosT], axis=0).astype(BF)

    # [tri01 | ma | mb | ident] constants (see _emit)
    kc = np.zeros((128, 512), np.float32)
    kc[:, 0:128] = np.triu(np.ones((128, 128), np.float32))
    for j in range(64):
        kc[j, 128 + j] = 1.0
        kc[64 + j, 128 + j] = 1.0
        kc[j, 256 + 64 + j] = -1.0
        kc[64 + j, 256 + 64 + j] = 1.0
    kc[:, 384:512] = np.eye(128, dtype=np.float32)
    kc = kc.astype(BF)

    def head_rows(W, heads, pad=0.0):
        rows = np.full((HD, C), pad, np.float32)
        for i, h in enumerate(heads):
            rows[i * D : (i + 1) * D] = W[h * D : (h + 1) * D]
        return rows

    in_maps = []
    for b in range(B):
        xtb = np.ascontiguousarray(x[b].T).astype(BF)  # [C, T]
        for heads in GROUPS:
            wq = np.ascontiguousarray(head_rows(Wq, heads, pad=0.01).T).astype(BF)
            wk = np.ascontiguousarray(head_rows(Wk, heads).T).astype(BF)
            wv = np.ascontiguousarray(head_rows(Wv, heads).T).astype(BF)
            # Wproj columns for these heads, transposed: [HD, C]
            wp = np.zeros((HD, C), np.float32)
            for i, h in enumerate(heads):
                wp[i * D : (i + 1) * D] = Wproj[:, h * D : (h + 1) * D].T
            in_maps.append(
                {"xt": xtb, "wqt": wq, "wkt": wk, "wvt": wv,
                 "wpt": wp.astype(BF), "cs": cs, "sc": sc, "kconsts": kc}
            )
    return in_maps


def _gather(results):
    y = np.zeros((B, T, C), np.float32)
    for b in range(B):
        for g in range(len(GROUPS)):
            y[b] += results[b * len(GROUPS) + g]["out"].astype(np.float32)
    return y


def _run(in_maps, trace=False):
    from concourse.bass_utils import run_bass_kernel_spmd

    nc = _build()
    return run_bass_kernel_spmd(
        nc, in_maps, core_ids=list(range(N_CORES)), trace=trace
    )


def kernel(x, cos, sin, Wq, Wk, Wv, Wproj):
    ins = _shard(
        np.asarray(x), np.asarray(cos), np.asarray(sin),
        np.asarray(Wq), np.asarray(Wk), np.asarray(Wv), np.asarray(Wproj),
    )
    res = _run(ins, trace=False)
    return _gather(res.results)


def run_traced(x, cos, sin, Wq, Wk, Wv, Wproj):
    ins = _shard(
        np.asarray(x), np.asarray(cos), np.asarray(sin),
        np.asarray(Wq), np.asarray(Wk), np.asarray(Wv), np.asarray(Wproj),
    )
    res = _run(ins, trace=True)
    return _gather(res.results), res


# revision 34
# speedup vs baseline: 1.0133x; 1.0072x over previous
"""Causal self-attention (RoPE + qk-RMS-norm) Trainium2 kernel.

Sharding: 8 cores = 2 batches x 4 head-groups (tensor-parallel over heads,
data-parallel over batch). Each core computes its head-group's attention and
a row-parallel partial of the output projection; the host sums the 4
per-group partials per batch (the all-reduce of row-parallel sharding).

Per-core layout: Q.T/K.T computed directly in [d, t] (no transposes),
V in [t, d]. Transposed flash attention: S.T = K @ Q.T so P.T feeds the
PV matmul directly; softmax has no max-subtraction (RMS-normed scores are
bounded by sqrt(D)); column sums via ones-matmul; 1/sum deferred to Y.T.
Tokens are processed in two causal passes (halves of T) to fit SBUF.

Schedule (v2): per head h the PE runs QKproj(h) -> attention(h-1) ->
rope/norm(h), so the rope-product chain (PSUM evict -> DVE/gpsimd products)
of head h hides under attention(h-1) instead of stalling the PE. V
projections are emitted as PE filler right where DMA/chain latency would
otherwise bubble (pass starts). The q-norm row is produced by a PE
transpose + SBUF->SBUF DMA gather (8 packets) instead of a DRAM bounce
(1024 4B packets), and both softmax-normalize broadcasts use gpsimd
partition_broadcast instead of PE ones-matmuls.
"""

import functools

import numpy as np

B, T, C, H, D = 2, 2048, 1280, 10, 128
EPS = 1e-5
NHL = 3  # head slots per core (padded)
N_CORES = 8
NHALF = 2  # causal passes over T
# per-batch head groups (4th group padded with zero heads)
GROUPS = [[0, 1, 2], [3, 4, 5], [6, 7, 8], [9]]


def _emit(nc, tile, mybir, T, C, D, NHL, eps):
    F32 = mybir.dt.float32
    BF16 = mybir.dt.bfloat16
    I32 = mybir.dt.int32
    ActF = mybir.ActivationFunctionType
    Alu = mybir.AluOpType
    CCH = C // 128  # contraction chunks
    TBN = T // 128  # 128-token blocks
    T2 = T // NHALF  # tokens per pass
    TB2 = T2 // 128
    Q42 = T2 // 512  # q supertiles per pass
    HD = NHL * D
    couts = []
    off = 0
    while off < C:
        w = min(512, C - off)
        couts.append((off, w))
        off += w

    xt = nc.dram_tensor("xt", [C, T], BF16, kind="ExternalInput")
    wqt = nc.dram_tensor("wqt", [C, HD], BF16, kind="ExternalInput")
    wkt = nc.dram_tensor("wkt", [C, HD], BF16, kind="ExternalInput")
    wvt = nc.dram_tensor("wvt", [C, HD], BF16, kind="ExternalInput")
    wpt = nc.dram_tensor("wpt", [HD, C], BF16, kind="ExternalInput")
    cs = nc.dram_tensor("cs", [D, T], BF16, kind="ExternalInput")
    sc = nc.dram_tensor("sc", [D, T], BF16, kind="ExternalInput")
    # host-precomputed constants: [tri01 | ma | mb | ident]
    kconsts = nc.dram_tensor("kconsts", [128, 512], BF16, kind="ExternalInput")
    out = nc.dram_tensor("out", [T, C], BF16, kind="ExternalOutput")

    from contextlib import ExitStack

    with ExitStack() as ctx:
        ctx.enter_context(nc.allow_low_precision(reason="bf16 operands"))
        tc = ctx.enter_context(tile.TileContext(nc))
        pool = lambda n, b, **kw: ctx.enter_context(tc.tile_pool(name=n, bufs=b, **kw))
        per = pool("persist", 1)
        wvp = pool("wv", 1)
        wqkp = pool("wqk", 1)
        wptp = pool("wpt", 1)
        xtp = pool("xt", 2)
        qtp = pool("qt", 2)
        qsp = pool("qs", 1)
        ytp = pool("yt", 2)
        tmp = pool("tmp", 2)
        sqp = pool("sqp", 1)
        ptp = pool("ptp", 3)
        rows = pool("rows", 2)
        oev = pool("oev", 2)
        bqp = pool("bqp", 2)
        psmm = pool("psmm", 2, space="PSUM")
        psacc = pool("psacc", 2, space="PSUM")
        psrow = pool("psrow", 2, space="PSUM")

        # ---- constants ----
        ones_f = per.tile([128, 128], F32, tag="onf")
        nc.vector.memset(ones_f[:], 1.0)
        ones_col = per.tile([128, 1], BF16, tag="onc")
        nc.scalar.copy(ones_col[:], ones_f[:, 0:1])
        # constants tile: [tri01 | ma | mb | ident] DMA'd from the host
        kcon = per.tile([128, 512], BF16, tag="kcon")
        nc.sync.dma_start(kcon[:], kconsts[:, :])
        tri01 = kcon[:, 0:128]
        ma = kcon[:, 128:256]
        mb = kcon[:, 256:384]
        ident = kcon[:, 384:512]

        # PE warm-up: dummy accumulating matmuls during the initial DMA ramp
        warm = nc.dram_tensor("warm", [1, 512], F32, kind="ExternalOutput")
        wrhs = per.tile([128, 512], BF16, tag="wrhs")
        nc.vector.memset(wrhs[:], 1.0)
        wps = psrow.tile([1, 512], F32, tag="row", name="warmps")
        NWARM = 24
        for i in range(NWARM):
            nc.tensor.matmul(
                wps[:], ones_col[:], wrhs[:], start=(i == 0), stop=(i == NWARM - 1)
            )
        wsb = rows.tile([1, 512], F32, tag="rw", name="warmsb")
        nc.vector.tensor_copy(wsb[:], wps[:])
        nc.sync.dma_start(warm[:], wsb[:])

        wv = []
        wqr = []
        wkr = []
        # V for all heads/all tokens: [tk-part, tb, h, d]
        v_t = per.tile([128, TBN, NHL, D], BF16, tag="v")
        # K.T per head, all tokens
        ktr = [per.tile([128, T], BF16, tag=f"ktr{h}", name=f"ktr{h}")
               for h in range(NHL)]
        rk_cols = [per.tile([128, TBN], F32, tag=f"rkc{h}", name=f"rkc{h}")
                   for h in range(NHL)]

        # output-projection weights (resident); loaded late (first needed at
        # the pass-0 output projection ~150us in) so the transfers don't
        # compete with the startup x/wv/wq/wk loads for HBM bandwidth
        wp = {}

        def load_wp():
            # on gpsimd (sync is reserved for the latency-critical rwr DMAs)
            for hh in range(NHL):
                for ci, (co, cw) in enumerate(couts):
                    t = wptp.tile([128, cw], BF16, tag=f"wp{hh}_{ci}")
                    nc.gpsimd.dma_start(
                        t[:], wpt[hh * 128 : (hh + 1) * 128, co : co + cw]
                    )
                    wp[(hh, ci)] = t

        def emit_qapply(qtn, rwr):
            """Deferred q-norm: broadcast the rsqrt row over partitions via
            gpsimd partition_broadcast, then scale qtn in place (all-bf16
            DVE muls run in 2x mode)."""
            bqt = bqp.tile([128, T2], BF16, tag="bqt")
            nc.gpsimd.partition_broadcast(bqt[:], rwr[:])
            for q4 in range(Q42):
                lsl = slice(q4 * 512, (q4 + 1) * 512)
                nc.vector.tensor_mul(qtn[:, lsl], qtn[:, lsl], bqt[:, lsl])

        def emit_attention(hf, h, qtn, ytn, mid_cb=None):
            """Attention for head h over this pass's q supertiles.
            kb-outer (K/V stationary reuse); st/exp run one kb ahead of
            PV/colsum so the in-order PE queue never waits on ACT.
            Normalize (1/colsum) is emitted inline per q4 as soon as its
            last PV lands -- pure DVE/gpsimd work, frees the PSUM
            accumulator immediately."""
            gq4s = [hf * Q42 + q4 for q4 in range(Q42)]
            csrs = []
            yts = [psacc.tile([128, 512], F32, tag="acc", name=f"yt{q4}")
                   for q4 in range(Q42)]

            def normalize_q4(q4):
                lsl = slice(q4 * 512, (q4 + 1) * 512)
                # reciprocal on the [1,512] row, then gpsimd broadcast
                rr = rows.tile([1, 512], F32, tag="rr", name="rr", bufs=2)
                nc.vector.reciprocal_approx_fast(rr[:], csrs[q4][:])
                rr8 = rows.tile([1, 512], BF16, tag="rr8", name="rr8", bufs=2)
                nc.vector.tensor_copy(rr8[:], rr[:])
                bcb = bqp.tile([128, 512], BF16, tag="bcb", bufs=2)
                nc.gpsimd.partition_broadcast(bcb[:], rr8[:])
                nc.vector.tensor_mul(ytn[:, h, lsl], yts[q4][:], bcb[:])
            # P column-sum accumulator (DVE bf16: 2x mode, light queue load)
            pacc = sqp.tile([128, Q42 * 512], BF16, tag="pacc", bufs=2)
            kbmax = 4 * (gq4s[-1] + 1)
            LA = 2  # st/exp run this many kb steps ahead of PV
            pts = {}  # kb -> pair pt tile awaiting PV
            for kb in range(kbmax + LA):
                if kb < kbmax:
                    active = [q4 for q4 in range(Q42) if kb <= 4 * gq4s[q4] + 3]
                    q0 = active[0]
                    j0 = kb - 4 * gq4s[q0]
                    st = psmm.tile([128, Q42 * 512], F32, tag="mm", name="st")
                    for q4 in active:
                        # strictly-above-diagonal q columns of the first
                        # active q4 are never needed: narrow the S matmul
                        a0 = q4 * 512 + (j0 * 128 if q4 == q0 and j0 > 0 else 0)
                        nc.tensor.matmul(
                            st[:, a0 : (q4 + 1) * 512],
                            ktr[h][:, kb * 128 : (kb + 1) * 128],
                            qtn[:, a0 : (q4 + 1) * 512],
                            start=True, stop=True,
                        )
                    pt = ptp.tile([128, Q42 * 512], BF16, tag="pt")
                    # one exp over the contiguous valid span of all active q4s
                    lo = q0 * 512 + (j0 * 128 if j0 > 0 else 0)
                    hi = (active[-1] + 1) * 512
                    nc.scalar.activation(
                        pt[:, lo:hi], st[:, lo:hi], ActF.Exp,
                        scale=rk_cols[h][:, kb : kb + 1],
                    )
                    if 0 <= j0 <= 3:
                        dg = slice(q0 * 512 + j0 * 128, q0 * 512 + (j0 + 1) * 128)
                        nc.vector.tensor_mul(pt[:, dg], pt[:, dg], tri01)
                    for q4 in active:
                        a0 = q4 * 512 + (j0 * 128 if q4 == q0 and j0 > 0 else 0)
                        lsl = slice(a0, (q4 + 1) * 512)
                        if kb == 0:
                            nc.vector.tensor_copy(pacc[:, lsl], pt[:, lsl])
                        else:
                            nc.vector.tensor_add(
                                pacc[:, lsl], pacc[:, lsl], pt[:, lsl]
                            )
                        # per-q4 column sum as soon as its pacc is final, so
                        # the pass-end chain isn't serialized behind the
                        # whole DVE queue
                        if kb == 4 * gq4s[q4] + 3:
                            csum = psrow.tile([1, 512], F32, tag="row",
                                              name=f"cs{q4}")
                            nc.tensor.matmul(
                                csum[:], ones_col[:],
                                pacc[:, q4 * 512 : (q4 + 1) * 512],
                                start=True, stop=True,
                            )
                            csr = rows.tile([1, 512], F32, tag="csr",
                                            name="csr", bufs=4)
                            nc.vector.tensor_copy(csr[:], csum[:])
                            csrs.append(csr)
                    pts[kb] = pt
                if kb == 4 and mid_cb is not None:
                    # deferred PE work (the next head's rsqrt transpose)
                    # slotted a few kb into this attention, when its DVE
                    # input has certainly landed
                    mid_cb()
                    mid_cb = None
                if kb >= LA:
                    pkb = kb - LA
                    pt = pts.pop(pkb)
                    for q4 in range(Q42):
                        gq4 = gq4s[q4]
                        last_kb = 4 * gq4 + 3
                        if pkb > last_kb:
                            continue
                        jp = pkb - 4 * gq4
                        w0 = jp * 128 if jp > 0 else 0
                        nc.tensor.matmul(
                            yts[q4][:, w0:],
                            v_t[:, pkb, h, :],
                            pt[:, q4 * 512 + w0 : (q4 + 1) * 512],
                            start=(pkb == 0), stop=(pkb == last_kb),
                        )
                        if pkb == last_kb:
                            normalize_q4(q4)

        pending = None  # deferred attention emitter for the previous head
        pending_qapply = None  # deferred q-norm apply for the previous head
        pending_oproj = None  # deferred output projection for the prev pass
        pending_transpose = None  # deferred rsqrt-row transpose

        def load_xc(hf_):
            # pass-1 prefetch: one full-width descriptor per chunk on
            # gpsimd (no urgency; fewer descriptors keeps gpsimd free for
            # the partition_broadcasts)
            toff_ = hf_ * T2
            xcl = []
            for c in range(CCH):
                t = xtp.tile([128, T2], BF16, tag=f"x{c}", name=f"x{c}")
                nc.gpsimd.dma_start(
                    t[:], xt[c * 128 : (c + 1) * 128, toff_ : toff_ + T2]
                )
                xcl.append(t)
            return xcl

        xc_next = None
        for hf in range(NHALF):
            toff = hf * T2
            # ---- per-pass cos/sin (stacked) ----
            cs_t = qtp.tile([D, T2], BF16, tag="cs", bufs=1)
            sc_t = qtp.tile([D, T2], BF16, tag="sc", bufs=1)
            nc.sync.dma_start(cs_t[:], cs[:, toff : toff + T2])
            nc.sync.dma_start(sc_t[:], sc[:, toff : toff + T2])
            # ---- x.T chunks: loaded here for pass 0, prefetched mid-pass-0
            # (double-buffered) for pass 1 ----
            if xc_next is not None:
                xc = xc_next
                xc_next = None
            else:
                # pass-0 startup, in consumption-priority order: x first
                # halves, wv, x second halves, wq, wk -- alternating the two
                # DMA-capable compute engines throughout.  wq after x second
                # halves: V-proj (first half) + warmup cover the ramp, and
                # QK-proj h0 only starts once both x halves are resident.
                xc = []
                for c in range(CCH):
                    t = xtp.tile([128, T2], BF16, tag=f"x{c}", name=f"x{c}")
                    xc.append(t)

                def xc_issue(half):
                    tsl = slice(half * (T2 // 2), (half + 1) * (T2 // 2))
                    for c in range(CCH):
                        (nc.gpsimd if c % 2 == 0 else nc.scalar).dma_start(
                            xc[c][:, tsl],
                            xt[c * 128 : (c + 1) * 128,
                               toff + half * (T2 // 2) :
                               toff + (half + 1) * (T2 // 2)],
                        )

                # wv interleaved with x half-0 (both needed by the first
                # V-proj blocks), then wq, wk, x half-1: QK-proj h0 emits
                # its q4-0 slices first, which need only x half-0
                tsl0 = slice(0, T2 // 2)
                for c in range(CCH):
                    nc.gpsimd.dma_start(
                        xc[c][:, tsl0],
                        xt[c * 128 : (c + 1) * 128, toff : toff + T2 // 2],
                    )
                    t = wvp.tile([128, HD], BF16, tag=f"wv{c}")
                    nc.scalar.dma_start(t[:], wvt[c * 128 : (c + 1) * 128, :])
                    wv.append(t)
                for c in range(CCH):
                    tq = wqkp.tile([128, HD], BF16, tag=f"wq{c}")
                    (nc.gpsimd if c % 2 == 0 else nc.scalar).dma_start(
                        tq[:], wqt[c * 128 : (c + 1) * 128, :]
                    )
                    wqr.append(tq)
                for c in range(CCH):
                    tk = wqkp.tile([128, HD], BF16, tag=f"wk{c}")
                    (nc.gpsimd if c % 2 == 0 else nc.scalar).dma_start(
                        tk[:], wkt[c * 128 : (c + 1) * 128, :]
                    )
                    wkr.append(tk)
                xc_issue(1)
                # dummy broadcast: loads the gpsimd custom-ISA microcode
                # library (~7us) during the DMA ramp, after the startup DMA
                # issues so descriptors aren't delayed behind it
                prime = bqp.tile([128, 512], BF16, tag="bcb")
                nc.gpsimd.partition_broadcast(prime[:], wrhs[0:1, :])

            def vproj_tbs(tb_lo, tb_hi, hf=hf, xc=xc):
                for tb in range(tb_lo, tb_hi):
                    gtb = hf * TB2 + tb
                    vp = psmm.tile([128, HD], F32, tag="mm", name="vp")
                    for c in range(CCH):
                        nc.tensor.matmul(
                            vp[:],
                            xc[c][:, tb * 128 : (tb + 1) * 128],
                            wv[c][:],
                            start=(c == 0), stop=(c == CCH - 1),
                        )
                    nc.vector.tensor_copy(v_t[:, gtb, :, :], vp[:])

            if hf == 0:
                # first half now; second half after QKproj(h0) as PE filler
                # while the h0 rope-product chain completes
                vproj_tbs(0, TB2 // 2)
                # second warm-up batch: soaks the remaining DMA ramp (wq
                # arrives after x half-0 + wv) and keeps the PE p-state high
                wps2 = psrow.tile([1, 512], F32, tag="row", name="warmps2")
                for i in range(12):
                    nc.tensor.matmul(
                        wps2[:], ones_col[:], wrhs[:], start=(i == 0),
                        stop=(i == 11),
                    )
                wsb2 = rows.tile([1, 512], F32, tag="rw", name="warmsb2")
                nc.vector.tensor_copy(wsb2[:], wps2[:])
                nc.sync.dma_start(warm[:], wsb2[:])

            # Y.T for this pass (all heads)
            ytn = ytp.tile([128, NHL, T2], BF16, tag="ytn")

            for h in range(NHL):
                # ---- Q/K projections into PSUM, evicted early to SBUF ----
                hds = slice(h * D, (h + 1) * D)
                qsb = {}
                qpst = {}
                for isq in range(2):
                    qpst[isq] = psmm.tile(
                        [128, Q42 * 512], F32, tag="mm", name="qps"
                    )
                # q4-0 slices of q then k first (they need only x half-0 at
                # startup), then the q4-1 slices; evict each slice as soon
                # as its chain completes
                for q4 in range(Q42):
                    if q4 == 1 and pending_transpose is not None:
                        # previous head's rsqrt transpose: mid-QKproj the
                        # PE reaches it ~4us in, when rqc has long landed,
                        # and the rwr->broadcast chain still finishes well
                        # before that head's attention needs it
                        pending_transpose()
                        pending_transpose = None
                    for isq, wt in enumerate((wqr, wkr)):
                        qps = qpst[isq]
                        for c in range(CCH):
                            nc.tensor.matmul(
                                qps[:, q4 * 512 : (q4 + 1) * 512],
                                wt[c][:, hds],
                                xc[c][:, q4 * 512 : (q4 + 1) * 512],
                                start=(c == 0), stop=(c == CCH - 1),
                            )
                        sb = qsp.tile([128, 512], BF16, tag=f"qs{isq}{q4}")
                        # q evictions on DVE (feed the critical rope->norm
                        # chain), k evictions on scalar to split queue load
                        if isq == 0:
                            nc.vector.tensor_copy(
                                sb[:], qps[:, q4 * 512 : (q4 + 1) * 512]
                            )
                        else:
                            nc.scalar.copy(
                                sb[:], qps[:, q4 * 512 : (q4 + 1) * 512]
                            )
                        qsb[(isq, q4)] = sb

                # PE filler between QKproj(h) and the attention/rope below:
                # V-proj second half (pass 0 also rides out the DMA ramp;
                # pass 1's first half was emitted at the pass-0 flush)
                if h == 0:
                    vproj_tbs(TB2 // 2, TB2)

                qtn = qtp.tile([128, T2], BF16, tag="qtn")

                # deferred q-norm apply for the previous head (its rsqrt row
                # landed during this head's QK projection), then its
                # attention; the rope-product chain for THIS head below
                # overlaps that attention on the PE
                if pending_qapply is not None:
                    pending_qapply()
                    pending_qapply = None

                # ---- rope products, phase A (emitted before the attention
                # so they sit early in the DVE/gpsimd queues) ----
                tprod = {}
                for isq in range(2):
                    for q4 in range(Q42):
                        # all products on DVE: gpsimd runs only DMA issues +
                        # partition_broadcast, so its custom-ISA microcode
                        # library is loaded once and never swapped (a swap
                        # costs ~6-7us of gpsimd downtime)
                        eng = nc.vector
                        qp = qsb[(isq, q4)]
                        lsl4 = slice(q4 * 512, (q4 + 1) * 512)
                        t1 = tmp.tile([128, 512], BF16, tag=f"t1{isq}{q4}")
                        t2 = tmp.tile([128, 512], BF16, tag=f"t2{isq}{q4}")
                        eng.tensor_mul(t1[:], qp[:], cs_t[:, lsl4])
                        eng.tensor_mul(t2[:], qp[:], sc_t[:, lsl4])
                        tprod[(isq, q4)] = (t1, t2)

                # bulk DMA-issue bursts go on gpsimd AFTER the qapply
                # broadcast is queued, and split across head sections so
                # no single burst delays the next head's bqt broadcast;
                # both after the startup ramp so the transfers don't
                # compete with the x/wq/wk loads for HBM bandwidth
                if hf == 0 and h == 1:
                    xc_next = load_xc(hf + 1)
                if hf == 0 and h == 2:
                    load_wp()

                # the pass-0 output projection runs here in pass-1 h0's
                # attention slot: its PE work covers the h0 rope-product
                # chain, and DVE saw h0's products queued first
                if pending_oproj is not None:
                    pending_oproj()
                    pending_oproj = None

                if pending is not None and h < NHL - 1:
                    pending()
                    pending = None

                # ---- rope + norm ----
                # Sum-of-squares lands as per-128-block COLUMNS (tiny N=1
                # matmuls), rsqrt is a quake-style bit-trick + 2 Newton steps
                # on DVE -- no scalar Sqrt/Ln, so the Exp act table is never
                # swapped out.
                nrm = psrow.tile([128, 2 * Q42 * 4], F32, tag="row", name="nrm")
                rope_io = [(qtn, 0), (ktr[h], toff)]
                for isq, (dst, doff) in enumerate(rope_io):
                    # phase B: rope matmuls + evictions
                    for q4 in range(Q42):
                        dsl = slice(doff + q4 * 512, doff + (q4 + 1) * 512)
                        t1, t2 = tprod[(isq, q4)]
                        rp = psmm.tile([128, 512], F32, tag="mm", name="rp")
                        nc.tensor.matmul(rp[:], ma, t1[:], start=True, stop=False)
                        nc.tensor.matmul(rp[:], mb, t2[:], start=False, stop=True)
                        nc.scalar.copy(dst[:, dsl], rp[:])
                    # phase C: squares (on ACT, right behind the rp evicts
                    # in its queue -- keeps DVE free for the rsqrt chain)
                    # + per-block column reduces
                    for q4 in range(Q42):
                        dsl = slice(doff + q4 * 512, doff + (q4 + 1) * 512)
                        sq = sqp.tile([128, 512], BF16, tag="sq")
                        nc.scalar.square(sq[:], dst[:, dsl])
                        for b in range(4):
                            co = isq * 8 + q4 * 4 + b
                            nc.tensor.matmul(
                                nrm[:, co : co + 1],
                                sq[:, b * 128 : (b + 1) * 128], ones_col[:],
                                start=True, stop=True,
                            )
                # rsqrt chain on [128, 16]: cols 0:8 = q (no eps; pad heads
                # get nonzero Wq host-side), cols 8:16 = k (ssk/D + eps)
                nsb = rows.tile([128, 16], F32, tag="nsb")
                nc.vector.tensor_copy(nsb[:, 0:8], nrm[:, 0:8])
                nc.vector.tensor_scalar(
                    nsb[:, 8:16], nrm[:, 8:16], 1.0 / D, float(eps),
                    op0=Alu.mult, op1=Alu.add,
                )
                ysb = rows.tile([128, 16], F32, tag="ysb")
                nsi = nsb[:].bitcast(I32)
                ysi = ysb[:].bitcast(I32)
                nc.vector.tensor_scalar(
                    ysi, nsi, 1, None, op0=Alu.logical_shift_right
                )
                nc.vector.tensor_scalar(
                    ysi, ysi, 0x5F3759DF, -1, op0=Alu.subtract, op1=Alu.mult
                )
                ntmp = rows.tile([128, 16], F32, tag="ntmp")
                for _ in range(2):
                    nc.vector.tensor_mul(ntmp[:], ysb[:], ysb[:])
                    nc.vector.tensor_mul(ntmp[:], ntmp[:], nsb[:])
                    nc.vector.tensor_scalar(
                        ntmp[:], ntmp[:], -0.5, 1.5, op0=Alu.mult, op1=Alu.add
                    )
                    nc.vector.tensor_mul(ysb[:], ysb[:], ntmp[:])
                # k: rsqrt columns drop straight into rk_cols (no transpose)
                nc.vector.tensor_copy(
                    rk_cols[h][:, hf * TB2 : (hf + 1) * TB2], ysb[:, 8:16]
                )
                rqc = rows.tile([128, 8], BF16, tag="rqc")
                nc.vector.tensor_copy(rqc[:], ysb[:, 0:8])

                # q: [128, 8] cols -> PE transpose -> [8, 128] -> local
                # SBUF->SBUF gather into a [1, T2] row (8 contiguous
                # packets).  Emission is deferred (pending_transpose) to a
                # PE-queue spot that is reached only after rqc has landed,
                # so the in-order PE never stalls waiting for the DVE chain.
                rwr_box = {}

                def emit_transpose(rqc=rqc, rwr_box=rwr_box):
                    rqt_ps = psrow.tile([8, 128], BF16, tag="row", name="rqt")
                    nc.tensor.matmul(
                        rqt_ps[:], rqc[:], ident, is_transpose=True,
                        start=True, stop=True,
                    )
                    rqt = rows.tile([8, 128], BF16, tag="rqts", name="rqts")
                    nc.vector.tensor_copy(rqt[:], rqt_ps[:])
                    rwr = rows.tile([1, T2], BF16, tag="rwr", bufs=2)
                    nc.sync.dma_start(
                        rwr[0:1, :].rearrange("a (j p) -> a j p", p=128),
                        rqt[:],
                    )
                    rwr_box["rwr"] = rwr

                # last head: its consumer (the flush attention) comes right
                # after this section, so slot the transpose into the middle
                # of the previous head's attention instead
                if h == NHL - 1:
                    if pending is not None:
                        pending(mid_cb=emit_transpose)
                        pending = None
                    else:
                        emit_transpose()
                else:
                    pending_transpose = emit_transpose

                pending_qapply = (lambda qtn=qtn, box=rwr_box:
                                  emit_qapply(qtn, box["rwr"]))
                pending = (lambda hf=hf, h=h, qtn=qtn, ytn=ytn, mid_cb=None:
                           emit_attention(hf, h, qtn, ytn, mid_cb))

            # ---- pass flush: last head's attention; the output projection
            # is deferred into the next pass's h0 attention slot (its PE
            # work covers that head's rope chain, and the eviction CASTs
            # queue on DVE behind the products instead of ahead of them)
            if xc_next is not None:
                # pass-1 V-proj first half: PE filler covering the last
                # head's broadcast chain
                vproj_tbs(0, TB2 // 2, hf=hf + 1, xc=xc_next)
            if pending_qapply is not None:
                pending_qapply()
                pending_qapply = None
            if pending is not None:
                pending()
                pending = None

            def emit_oproj(ytn=ytn, toff=toff):
                for tb in range(TB2):
                    # fused eviction tile: one output DMA per token block
                    # (48 descriptors/pass -> 16, off the sync engine)
                    ot = oev.tile([128, C], BF16, tag="otb")
                    for ci, (co, cw) in enumerate(couts):
                        op = psacc.tile([128, cw], F32, tag="acc", name="op")
                        for hh in range(NHL):
                            nc.tensor.matmul(
                                op[:],
                                ytn[:, hh, tb * 128 : (tb + 1) * 128],
                                wp[(hh, ci)][:],
                                start=(hh == 0), stop=(hh == NHL - 1),
                            )
                        if (tb * len(couts) + ci) % 2 == 0:
                            nc.vector.tensor_copy(ot[:, co : co + cw], op[:])
                        else:
                            nc.scalar.copy(ot[:, co : co + cw], op[:])
                    nc.gpsimd.dma_start(
                        out[toff + tb * 128 : toff + (tb + 1) * 128, :], ot[:]
                    )

            pending_oproj = emit_oproj
        if pending_oproj is not None:
            pending_oproj()
    return nc


@functools.lru_cache(maxsize=4)
def _build(T_=T, C_=C, D_=D, NHL_=NHL, eps=EPS):
    import concourse.bacc as bacc
    import concourse.tile as tile
    from concourse import mybir

    nc = bacc.Bacc("TRN2", target_bir_lowering=False)
    _emit(nc, tile, mybir, T_, C_, D_, NHL_, eps)
    nc.compile()
    return nc


def _shard(x, cos, sin, Wq, Wk, Wv, Wproj):
    """Build the 8 per-core input maps."""
    import ml_dtypes

    BF = ml_dtypes.bfloat16
    HD = NHL * D
    cosT = np.ascontiguousarray(cos[0, 0].T.astype(np.float32))  # [64, T]
    sinT = np.ascontiguousarray(sin[0, 0].T.astype(np.float32))
    cs = np.concatenate([cosT, sinT], axis=0).astype(BF)  # [128, T]
    sc = np.concatenate([sinT, cosT], axis=0).astype(BF)

    # [tri01 | ma | mb | ident] constants (see _emit)
    kc = np.zeros((128, 512), np.float32)
    kc[:, 0:128] = np.triu(np.ones((128, 128), np.float32))
    for j in range(64):
        kc[j, 128 + j] = 1.0
        kc[64 + j, 128 + j] = 1.0
        kc[j, 256 + 64 + j] = -1.0
        kc[64 + j, 256 + 64 + j] = 1.0
    kc[:, 384:512] = np.eye(128, dtype=np.float32)
    kc = kc.astype(BF)

    def head_rows(W, heads, pad=0.0):
        rows = np.full((HD, C), pad, np.float32)
        for i, h in enumerate(heads):
            rows[i * D : (i + 1) * D] = W[h * D : (h + 1) * D]
        return rows

    in_maps = []
    for b in range(B):
        xtb = np.ascontiguousarray(x[b].T).astype(BF)  # [C, T]
        for heads in GROUPS:
            wq = np.ascontiguousarray(head_rows(Wq, heads, pad=0.01).T).astype(BF)
            wk = np.ascontiguousarray(head_rows(Wk, heads).T).astype(BF)
            wv = np.ascontiguousarray(head_rows(Wv, heads).T).astype(BF)
            # Wproj columns for these heads, transposed: [HD, C]
            wp = np.zeros((HD, C), np.float32)
            for i, h in enumerate(heads):
                wp[i * D : (i + 1) * D] = Wproj[:, h * D : (h + 1) * D].T
            in_maps.append(
                {"xt": xtb, "wqt": wq, "wkt": wk, "wvt": wv,
                 "wpt": wp.astype(BF), "cs": cs, "sc": sc, "kconsts": kc}
            )
    return in_maps


def _gather(results):
    y = np.zeros((B, T, C), np.float32)
    for b in range(B):
        for g in range(len(GROUPS)):
            y[b] += results[b * len(GROUPS) + g]["out"].astype(np.float32)
    return y


def _run(in_maps, trace=False):
    from concourse.bass_utils import run_bass_kernel_spmd

    nc = _build()
    return run_bass_kernel_spmd(
        nc, in_maps, core_ids=list(range(N_CORES)), trace=trace
    )


def kernel(x, cos, sin, Wq, Wk, Wv, Wproj):
    ins = _shard(
        np.asarray(x), np.asarray(cos), np.asarray(sin),
        np.asarray(Wq), np.asarray(Wk), np.asarray(Wv), np.asarray(Wproj),
    )
    res = _run(ins, trace=False)
    return _gather(res.results)


def run_traced(x, cos, sin, Wq, Wk, Wv, Wproj):
    ins = _shard(
        np.asarray(x), np.asarray(cos), np.asarray(sin),
        np.asarray(Wq), np.asarray(Wk), np.asarray(Wv), np.asarray(Wproj),
    )
    res = _run(ins, trace=True)
    return _gather(res.results), res
